# revision 56
# baseline (speedup 1.0000x reference)
"""Trainium2 Bass kernel for the dual-softmax cross-attention module (fp8 V2).

Sharding: 8 cores = batch (4) x head-half (2).  Core c handles batch c//2 and
heads 4*(c%2) .. 4*(c%2)+4, producing a disjoint 256-channel slice of both
outputs after a 2-core AllGather of context halves.

Engine plan (per core):
- All hot matmuls run fp8e4m3 with DoubleRow perf mode (2 k-subtiles per
  instruction, 0.5 cycles/row): Q/K/V projections, scores, ctx1, ctx2.
- Host-side scaling keeps every fp8 tensor in the normal range: Wq/Wk x16
  (undone in the exp scale), Wv1/Wv2 x1024 (undone in Wo1/Wo2).
- exp(s) runs on the ACT engine in two [128,1024] chunks per query tile with
  fused accumulator rowsums (no DVE reduces).  E is stored fp8.
- E^T is produced by uint16 XBAR transposes of the fp8 E (adjacent k pairs
  land interleaved on one partition); ctx1 consumes the pair dim as its
  DoubleRow k-subtiles, with V1 projected in matching parity-interleaved
  token order (stride-2 token APs, free).
- ctx2 accumulates over query-tile pairs with a ones-column appended to the
  scaled V2 (colsum for free); rowsum/colsum reciprocals scale V instead of E.
- Per-head epilogue (ctx2 halves, colsum, ctx1 kb-bursts) runs interleaved
  into the next head's query loop, sized to exactly 8 PSUM banks.
"""

import sys

for _p in ("/opt/trn_rl_repo", "/opt/pypackages"):
    if _p not in sys.path:
        sys.path.insert(0, _p)

import numpy as np
import ml_dtypes

import concourse.bass as bass
import concourse.tile as tile
from concourse import bacc, mybir
from concourse.bass_utils import run_bass_kernel_spmd

F32 = mybir.dt.float32
BF16 = mybir.dt.bfloat16
FP8 = mybir.dt.float8e4
U16 = mybir.dt.uint16
AF = mybir.ActivationFunctionType
AX = mybir.AxisListType
DR = mybir.MatmulPerfMode.DoubleRow

N_CORES = 8
B = 4          # batch
C = 512        # channels
N = 2048       # tokens (8*16*16)
H = 8          # heads
DH = 64        # head dim
HL = 4         # heads per core
CL = 256       # channels per core (head-group)
NT = N // 128  # 16 token tiles
CT = C // 128  # 4 channel tiles
KB = 8         # k-blocks of 256 tokens (parity pairs on 128 partitions)

QK_SCALE = 16.0       # host scale on Wq/Wk (fp8 subnormal dodge)
V_SCALE = 1024.0      # host scale on Wv1/Wv2 (descaled in the output evac)
WO_SCALE = 8.0        # host scale on Wo1/Wo2 (fp8 subnormal dodge)
EXP_SCALE = 0.125 / (QK_SCALE * QK_SCALE)

_BF = ml_dtypes.bfloat16
_F8 = ml_dtypes.float8_e4m3


def _build():
    nc = bacc.Bacc("TRN2", target_bir_lowering=False, debug=False,
                   num_devices=N_CORES)

    def din(name, shape, dt=FP8):
        return nc.dram_tensor(name, shape, dt, kind="ExternalInput").ap()

    x1b = din("x1b", [CT, 128, N])          # x1[b] channel-major, fp8
    x2b = din("x2b", [CT, 128, N])
    wqk = din("wqk", [128, 2, CT, CL])      # Wq/Wk cols permuted, x16, fp8
    wv = din("wv", [128, 2, CT, CL])        # Wv1/Wv2 cols, x1024, fp8
    wo = din("wo", [128, 2, CT, CL])        # Wo2/Wo1 cols x8, fp8
    bqk = din("bqk", [64, 8, 1], F32)       # q/k bias, permuted, x16
    bv = din("bv", [1, 2 * CL], BF16)       # v1/v2 bias x1024, bf16
    x1r = din("x1r", [2, 128, N], F32)      # residual slice + bo1
    x2r = din("x2r", [2, 128, N], F32)

    o1 = nc.dram_tensor("o1", [2, 128, N], F32, kind="ExternalOutput").ap()
    o2 = nc.dram_tensor("o2", [2, 128, N], F32, kind="ExternalOutput").ap()

    with tile.TileContext(nc) as tc:
        _emit(nc, tc, locals())
    nc.compile()
    return nc


def _emit(nc, tc, t):
    x1b, x2b = t["x1b"], t["x2b"]
    wqk, wv, wo, bqk, bv = t["wqk"], t["wv"], t["wo"], t["bqk"], t["bv"]
    x1r, x2r, o1, o2 = t["x1r"], t["x2r"], t["o1"], t["o2"]

    from contextlib import ExitStack
    ctx = ExitStack()
    with ctx:
        persist = ctx.enter_context(tc.tile_pool(name="persist", bufs=1))
        small = ctx.enter_context(tc.tile_pool(name="small", bufs=8))
        dram = ctx.enter_context(tc.tile_pool(name="dram", bufs=2, space="DRAM"))

        # ---- persistent SBUF ----
        w8 = persist.tile([128, 4, CT, CL], FP8, tag="w8")       # q,k,v1,v2
        wo_s = persist.tile([128, 2, CT, CL], FP8, tag="wo")     # x8 scale
        bqk_s = persist.tile([64, 8, 1], F32, tag="bqk")
        bv_s = persist.tile([1, 2 * CL], BF16, tag="bv")
        bvb = persist.tile([128, 2 * CL], BF16, tag="bvb")  # broadcast rows
        onec = persist.tile([128, 1], BF16, tag="onec")  # bf16 ones col
        oner = persist.tile([1, 128], BF16, tag="oner")  # bf16 ones row
        # Q/K: [hl_loc*32+d32, tensor, head-pair, d-half, tok] (base 0/32)
        qk2 = persist.tile([64, 2, 2, 2, N], FP8, tag="qk2")
        v2tok = persist.tile([128, NT, CL], BF16, tag="v2tok")   # token-major
        v1tok = persist.tile([128, KB, 2, CL], BF16, tag="v1tok")  # parity
        et = persist.tile([128, KB, N], U16, tag="et")   # E^T, k-pairs per part
        cm = {}

        nc.gpsimd.dma_start(w8[:, 0:2, :, :], wqk[:, :, :, :])
        nc.gpsimd.dma_start(bqk_s[:, :, :], bqk[:, :, :])
        nc.gpsimd.dma_start(w8[:, 2:4, :, :], wv[:, :, :, :])
        nc.gpsimd.dma_start(bv_s[:, :], bv[:, :])
        nc.gpsimd.dma_start(wo_s[:, :, :, :], wo[:, :, :, :])
        nc.vector.memset(onec[:, :], 1.0)
        nc.vector.memset(oner[:, :], 1.0)

        # ---- P2 SBUF slab pools ----
        p2 = ExitStack()
        es_pool = p2.enter_context(tc.tile_pool(name="es", bufs=2))
        v2p_pool = p2.enter_context(tc.tile_pool(name="v2p", bufs=2))
        v1p_pool = p2.enter_context(tc.tile_pool(name="v1p", bufs=2))
        gs_pool = p2.enter_context(tc.tile_pool(name="gs", bufs=2))
        cs_pool = p2.enter_context(tc.tile_pool(name="cs", bufs=2))

        # ---- P1: x loads + Q/K projections (fp8 DR) ----
        xb_stack = ExitStack()
        xt_pool = xb_stack.enter_context(tc.tile_pool(name="xt", bufs=1))
        xt = xt_pool.tile([128, 2, CT, N], FP8, tag="xt")
        # x tiles on the scalar queue (idle until attention) so they overlap
        # the weight loads on gpsimd; ti 0,1 of both streams land first
        for ti in range(CT):
            for s, xsrc in ((0, x1b), (1, x2b)):
                nc.scalar.dma_start(xt[:, s, ti, :], xsrc[ti, :, :])

        pj_stack = ExitStack()
        pj_ps = pj_stack.enter_context(
            tc.tile_pool(name="pj_ps", bufs=4, space="PSUM"))
        for ti_t, s in ((0, 0), (1, 1)):
            for ch in range(2):
                for half in range(2):
                    for u in range(4):
                        ps = pj_ps.tile([64, 512], F32, tag="pj",
                                        name=f"pj{ti_t}_{ch}_{half}_{u}")
                        co = half * 128 + ch * 64
                        for tp in (0, 2):
                            nc.tensor.matmul(
                                ps[:, :],
                                w8[:, ti_t, tp:tp + 2, co:co + 64],
                                xt[:, s, tp:tp + 2, u * 512:(u + 1) * 512],
                                start=(tp == 0), stop=(tp == 2), perf_mode=DR)
                        nc.vector.tensor_scalar_add(
                            qk2[0:64, ti_t, ch, half, u * 512:(u + 1) * 512],
                            ps[:, :],
                            bqk_s[0:64, 4 * ti_t + 2 * ch + half, :])

        # broadcast bv to all 128 partitions via a K=1 matmul (emitted after
        # the Q/K projections so it doesn't block the PE queue head at start)
        bps = pj_ps.tile([128, 2 * CL], F32, tag="bvb", name="bvb_ps")
        nc.tensor.matmul(bps[:, :], oner[:, :], bv_s[:, :],
                         start=True, stop=True)
        nc.vector.tensor_copy(bvb[:, :], bps[:, :])
        pj_stack.close()

        # PSUM pools for P2 (opened only after pj frees its banks).
        # PSUM pools pop LIFO: sc_ps outlives vps, epi_ps opens after vps.
        sc_ps = p2.enter_context(tc.tile_pool(name="sc_ps", bufs=3, space="PSUM"))
        vps_stack = ExitStack()
        vps_pool = vps_stack.enter_context(
            tc.tile_pool(name="vps", bufs=2, space="PSUM"))
        epi_ps = None  # opened once vps closes (end of head 0)

        def emit_v2_proj(qt):
            ps = vps_pool.tile([128, 256], F32, tag="vps", name=f"v2ps{qt}")
            for tp in (0, 2):
                nc.tensor.matmul(
                    ps[:, 0:CL],
                    xt[:, 1, tp:tp + 2, qt * 128:(qt + 1) * 128],
                    w8[:, 3, tp:tp + 2, :],
                    start=(tp == 0), stop=(tp == 2), perf_mode=DR)
            nc.vector.tensor_add(v2tok[:, qt, :], ps[:, 0:CL],
                                 bvb[:, CL:2 * CL])

        def emit_v1_proj(kb, par):
            # parity-interleaved token order: partition j <- token kb*256+2j+par
            ps = vps_pool.tile([128, 256], F32, tag="vps", name=f"v1ps{kb}_{par}")
            xpar = xt[:, 0, :, :].rearrange(
                "p t (kb j two) -> p t kb two j", kb=KB, j=128, two=2)
            for tp in (0, 2):
                nc.tensor.matmul(
                    ps[:, 0:CL], xpar[:, tp:tp + 2, kb, par, :],
                    w8[:, 2, tp:tp + 2, :],
                    start=(tp == 0), stop=(tp == 2), perf_mode=DR)
            nc.vector.tensor_add(v1tok[:, kb, par, :], ps[:, 0:CL],
                                 bvb[:, 0:CL])

        # ---- P2: per-head attention, epilogue of head hl-1 interleaved ----
        st = {}

        def emit_scores_exp(hl, qt):
            s = st[hl]
            lo, hp = (hl % 2) * 32, hl // 2
            for c in range(2):
                ps = sc_ps.tile([128, 1024], F32, tag="sc", name=f"sc{qt}_{c}")
                for u in (2 * c, 2 * c + 1):
                    nc.tensor.matmul(
                        ps[:, (u % 2) * 512:(u % 2) * 512 + 512],
                        qk2[lo:lo + 32, 0, hp, :, qt * 128:(qt + 1) * 128],
                        qk2[lo:lo + 32, 1, hp, :, u * 512:(u + 1) * 512],
                        start=True, stop=True, perf_mode=DR)
                nc.scalar.activation(
                    s["es"][:, qt, c * 1024:(c + 1) * 1024], ps[:, :],
                    AF.Exp, scale=EXP_SCALE,
                    accum_out=s["rsp"][:, 2 * qt + c:2 * qt + c + 1])

        def emit_rownorm(hl, qt):
            s = st[hl]
            sq = small.tile([128, 8], F32, tag="sq", bufs=4, name=f"sq{hl}_{qt}")
            rs, rr = sq[:, 0:1], sq[:, 1:2]
            nc.vector.reduce_sum(out=rs[:, :], in_=s["rsp"][:, 2 * qt:2 * qt + 2],
                                 axis=AX.X)
            nc.vector.reciprocal(rr[:, :], rs[:, :])
            nc.vector.tensor_scalar_mul(
                s["v2p"][:, qt, 0:DH],
                v2tok[:, qt, hl * DH:(hl + 1) * DH], rr[:, :])

        def emit_ctx2(hl, pair, half):
            # 2 DR instrs accumulating a query-tile pair into a k-half
            s = st[hl]
            for ch in range(2):
                o = half * 1024 + ch * 512
                nc.tensor.matmul(
                    s["c2"][half][0:DH + 1, ch * 512:(ch + 1) * 512],
                    s["v2p"][:, 2 * pair:2 * pair + 2, 0:DH + 1],
                    s["es"][:, 2 * pair:2 * pair + 2, o:o + 512],
                    start=(pair == 0), stop=(pair == NT // 2 - 1), perf_mode=DR)

        def emit_ctx2_evac(hl, half):
            s = st[hl]
            nc.vector.tensor_copy(s["csrow"][0:1, half * 1024:(half + 1) * 1024],
                                  s["c2"][half][DH:DH + 1, :])
            nc.vector.tensor_copy(s["gs"][0:DH, half * 1024:(half + 1) * 1024],
                                  s["c2"][half][0:DH, :])

        def emit_colsum(hl):
            # colsum row -> parity-ordered columns via 16 K=1 matmuls
            s = st[hl]
            cs_ps = epi_ps.tile([128, 16], F32, tag="epi", name=f"csps{hl}")
            csp = s["csrow"].rearrange(
                "p (kb j two) -> p kb two j", kb=KB, j=128, two=2)
            for kb in range(KB):
                for par in range(2):
                    nc.tensor.matmul(cs_ps[:, 2 * kb + par:2 * kb + par + 1],
                                     csp[0:1, kb, par, :], onec[0:1, :],
                                     start=True, stop=True)
            cr = small.tile([128, 16], F32, tag="cr", bufs=2, name=f"cr{hl}")
            s["cr"] = cr
            nc.vector.reciprocal(cr[:, :], cs_ps[:, :])

        def emit_v1p(hl, kb):
            s = st[hl]
            for par in range(2):
                nc.vector.tensor_scalar_mul(
                    s["v1p"][:, kb, par, :],
                    v1tok[:, kb, par, hl * DH:(hl + 1) * DH],
                    s["cr"][:, 2 * kb + par:2 * kb + par + 1])

        # ctx1 over the u16-transposed E^T: even-parity tokens (byte 0 of each
        # u16 pair, 2B-aligned start) use DoubleRow with kb-region pairs
        # (16KB-aligned subtile stride); odd-parity tokens start at an odd
        # byte, which DoubleRow forbids, so they run as plain fp8 matmuls.
        et8v = et.bitcast(FP8).rearrange(
            "p (g kbs) (q two) -> p kbs g two q", g=2, kbs=KB // 2, two=2)
        et8k = et.bitcast(FP8).rearrange(
            "p kb (q two) -> p kb two q", two=2)

        def emit_ctx1(hl, qc, part):
            # part 0: even-parity DR (4 instrs) + odd kb 0-3; part 1: odd kb 4-7
            # Each qc accumulates in its own 1-bank psum tile, evacuated
            # immediately (keeps the epilogue pool at 2 banks so sc_ps can
            # triple-buffer).
            s = st[hl]
            v1pv = s["v1p"].rearrange("p (g kbs) two d -> p kbs g two d", g=2)
            if part == 0:
                s["c1"] = epi_ps.tile([DH, 512], F32, tag="epi",
                                      name=f"c1_{hl}_{qc}")
            dst = s["c1"][0:DH, :]
            if part == 0:
                for kbs in range(KB // 2):
                    nc.tensor.matmul(
                        dst, v1pv[:, kbs, :, 0, :],
                        et8v[:, kbs, :, 0, qc * 512:(qc + 1) * 512],
                        start=(kbs == 0), stop=False, perf_mode=DR)
                kbr = range(0, KB // 2)
            else:
                kbr = range(KB // 2, KB)
            for kb in kbr:
                nc.tensor.matmul(
                    dst, s["v1p"][:, kb, 1, :],
                    et8k[:, kb, 1, qc * 512:(qc + 1) * 512],
                    start=False, stop=(part == 1 and kb == KB - 1))
            if part == 1:
                nc.vector.tensor_copy(
                    s["gs"][DH:128, qc * 512:(qc + 1) * 512], dst)

        def emit_transpose(hl, qt):
            # one instr transposes all 8 kb chunks: out[p, kb, q] 3D AP.
            # Alternate queues so the per-head transpose chain halves.
            s = st[hl]
            eng = nc.sync
            eng.dma_start(
                et[:, :, qt * 128:(qt + 1) * 128],
                s["es"][:, qt, :].bitcast(U16),
                transpose=True)

        def emit_gather(hl, half=None):
            s = st[hl]
            poff = 64 * (hl % 2)
            rows = slice(0, 128) if half is None else (
                slice(0, 64) if half == 0 else slice(64, 128))
            nr = rows.stop - rows.start
            sfx = f"{hl}_{half}"
            gin = dram.tile([nr, N], FP8, tag="gin", name=f"gin{sfx}")
            gout = dram.tile([2, nr, N], FP8, tag="gout", bufs=4,
                             name=f"gout{sfx}")
            nc.gpsimd.dma_start(gin[:, :], s["gs"][rows, :])
            nc.gpsimd.collective_compute(
                "AllGather", mybir.AluOpType.bypass,
                replica_groups=[[0, 1], [2, 3], [4, 5], [6, 7]],
                ins=[gin.opt()], outs=[gout.opt()])
            for r in range(2):
                tt = 2 * r + hl // 2
                if half in (None, 0):
                    nc.gpsimd.dma_start(cm["2"][poff:poff + 64, tt, :],
                                        gout[r, 0:64, :])
                if half in (None, 1):
                    ro = 64 if half is None else 0
                    nc.gpsimd.dma_start(cm["1"][poff:poff + 64, tt, :],
                                        gout[r, ro:ro + 64, :])

        CTX2_SLOTS = {1: (0, 0, 3), 2: (3, 0, 3), 3: (6, 0, 2),
                      4: (0, 1, 3), 5: (3, 1, 3), 6: (6, 1, 2)}

        def emit_epilogue_piece(hl, qt):
            # epilogue of head hl scheduled into head hl+1's qt slot
            if qt in CTX2_SLOTS:
                p0, half, n = CTX2_SLOTS[qt]
                for pair in range(p0, p0 + n):
                    emit_ctx2(hl, pair, half)
                if p0 + n == KB:
                    emit_ctx2_evac(hl, half)
            elif qt == 7:
                emit_colsum(hl)
                for kb in range(KB):
                    emit_v1p(hl, kb)
            elif 8 <= qt <= 15:  # ctx1: 8 balanced pieces
                emit_ctx1(hl, (qt - 8) // 2, (qt - 8) % 2)

        def finish_ctx1(hl):
            emit_gather(hl)

        def new_head_state(hl):
            st[hl] = {
                "es": es_pool.tile([128, NT, N], FP8, tag="es", name=f"es{hl}"),
                "c2": {}, "c1": None, "cr": None,
                "rsp": small.tile([128, 32], F32, tag="rsp", bufs=2,
                                  name=f"rsp{hl}"),
                "gs": gs_pool.tile([128, N], FP8, tag="gs", name=f"gs{hl}"),
                "csrow": cs_pool.tile([1, N], BF16, tag="csr", name=f"csr{hl}"),
            }
            # 80B row stride: DoubleRow subtile strides must be 16B-aligned
            v2p = v2p_pool.tile([128, NT, 80], FP8, tag="v2p",
                                name=f"v2p{hl}")
            st[hl]["v2p"] = v2p
            nc.vector.memset(v2p[:, :, DH:DH + 1], 1.0)
            st[hl]["v1p"] = v1p_pool.tile([128, KB, 2, DH], FP8, tag="v1p",
                                          name=f"v1p{hl}")

        for hl in range(HL):
            new_head_state(hl)
            if hl > 0:
                st[hl - 1]["c2"][0] = epi_ps.tile([DH + 1, 1024], F32, tag="epi",
                                                  name=f"c2a{hl - 1}")
            for qt in range(NT):
                emit_scores_exp(hl, qt)
                if hl > 0:
                    emit_epilogue_piece(hl - 1, qt)
                    if qt == 3:
                        st[hl - 1]["c2"][1] = epi_ps.tile(
                            [DH + 1, 1024], F32, tag="epi", name=f"c2b{hl - 1}")
                else:
                    emit_v2_proj(qt)
                    emit_v1_proj(qt // 2, qt % 2)
                emit_rownorm(hl, qt)
                # own transposes start as ctx1(hl-1) frees et q-ranges:
                # tq 0..11 during slots 10..15, tq 12..15 spill to the next
                # head's slots 0..1 (or the post-loop for the last head)
                if qt >= 10:
                    emit_transpose(hl, 2 * (qt - 10))
                    emit_transpose(hl, 2 * (qt - 10) + 1)
                if hl > 0 and qt < 2:
                    emit_transpose(hl - 1, 12 + 2 * qt)
                    emit_transpose(hl - 1, 12 + 2 * qt + 1)
            if hl == 0:
                vps_stack.close()
                xb_stack.close()
                cm_pool = p2.enter_context(tc.tile_pool(name="cm", bufs=1))
                cm["1"] = cm_pool.tile([128, CT, N], FP8, tag="cm1", name="cm1")
                cm["2"] = cm_pool.tile([128, CT, N], FP8, tag="cm2", name="cm2")
                epi_ps = p2.enter_context(
                    tc.tile_pool(name="epi_ps", bufs=1, space="PSUM"))
            if hl > 0:
                finish_ctx1(hl - 1)

        # ---- last head's epilogue + transposes ----
        hl = HL - 1
        s = st[hl]
        s["c2"][0] = epi_ps.tile([DH + 1, 1024], F32, tag="epi", name=f"c2a{hl}")
        for pair in range(NT // 2):
            emit_ctx2(hl, pair, 0)
        emit_ctx2_evac(hl, 0)
        s["c2"][1] = epi_ps.tile([DH + 1, 1024], F32, tag="epi", name=f"c2b{hl}")
        for pair in range(NT // 2):
            emit_ctx2(hl, pair, 1)
        for tq in range(12, NT):
            emit_transpose(hl, tq)
        emit_ctx2_evac(hl, 1)
        emit_gather(hl, half=0)
        emit_colsum(hl)
        for kb in range(KB):
            emit_v1p(hl, kb)
        # ctx1 with per-chunk gathers so the final AllGather pipelines
        # behind the remaining ctx1 chunks and P3 starts per-column
        for qc in range(4):
            emit_ctx1(hl, qc, 0)
            emit_ctx1(hl, qc, 1)
            poff = 64 * (hl % 2)
            gin = dram.tile([64, 512], FP8, tag="ginq", name=f"ginq{qc}")
            gout = dram.tile([2, 64, 512], FP8, tag="goutq", bufs=4,
                             name=f"goutq{qc}")
            nc.gpsimd.dma_start(gin[:, :],
                                s["gs"][DH:128, qc * 512:(qc + 1) * 512])
            nc.gpsimd.collective_compute(
                "AllGather", mybir.AluOpType.bypass,
                replica_groups=[[0, 1], [2, 3], [4, 5], [6, 7]],
                ins=[gin.opt()], outs=[gout.opt()])
            for r in range(2):
                tt = 2 * r + hl // 2
                nc.gpsimd.dma_start(
                    cm["1"][poff:poff + 64, tt, qc * 512:(qc + 1) * 512],
                    gout[r, :, :])

        p2.close()

        # ---- P3: output projections + residual ----
        p3 = ExitStack()
        o_ps = p3.enter_context(tc.tile_pool(name="o_ps", bufs=2, space="PSUM"))
        xr_pool = p3.enter_context(tc.tile_pool(name="xr", bufs=2))
        out_pool = p3.enter_context(tc.tile_pool(name="outp", bufs=2))
        for oi, (cmt, xr, oo) in enumerate(((cm["2"], x2r, o2),
                                            (cm["1"], x1r, o1))):
            w_s = wo_s[:, oi, :, :]
            cmv = cmt.rearrange("p (g ti) n -> p ti g n", g=2)
            wv_ = w_s.rearrange("p (g ti) c -> p ti g c", g=2)
            for m in range(2):
                xr_t = xr_pool.tile([128, N], F32, tag="xr")
                nc.gpsimd.dma_start(xr_t[:, :], xr[m, :, :])
                ps = o_ps.tile([128, N], F32, tag="o")
                # DR over cm-tile pairs (0,2) then (1,3): heads 0-1/4-5 are
                # gathered two heads early, so the first half starts sooner
                for ti in (0, 1):
                    for ch in range(4):
                        nc.tensor.matmul(
                            ps[:, ch * 512:(ch + 1) * 512],
                            wv_[:, ti, :, m * 128:(m + 1) * 128],
                            cmv[:, ti, :, ch * 512:(ch + 1) * 512],
                            start=(ti == 0), stop=(ti == 1), perf_mode=DR)
                ot = out_pool.tile([128, N], F32, tag="ot")
                nc.vector.scalar_tensor_tensor(
                    ot[:, :], ps[:, :], 1.0 / (V_SCALE * WO_SCALE), xr_t[:, :],
                    op0=mybir.AluOpType.mult, op1=mybir.AluOpType.add)
                nc.scalar.dma_start(oo[m, :, :], ot[:, :])
        p3.close()


_NC_CACHE = None


def _get_nc():
    global _NC_CACHE
    if _NC_CACHE is None:
        _NC_CACHE = _build()
    return _NC_CACHE


def _in_maps(x1, x2, Wq, bq, Wk, bk, Wv1, bv1, Wv2, bv2, Wo1, bo1, Wo2, bo2):
    x1f = np.asarray(x1, np.float32).reshape(B, C, N)
    x2f = np.asarray(x2, np.float32).reshape(B, C, N)

    # permuted column order for Wq/Wk: j = half*128 + ch*64 + hl_loc*32 + d32
    j = np.arange(CL)
    half, r = j // 128, j % 128
    chp, r2 = r // 64, r % 64
    colperm = (2 * chp + r2 // 32) * 64 + half * 32 + (r2 % 32)

    in_maps = []
    for c in range(N_CORES):
        b, hq = c // 2, c % 2
        sl = slice(CL * hq, CL * hq + CL)

        def wslice(W, scale, perm=None, dt=_F8):
            w = np.asarray(W, np.float32)[:, sl] * scale
            if perm is not None:
                w = w[:, perm]
            return np.ascontiguousarray(
                w.reshape(CT, 128, CL).transpose(1, 0, 2)).astype(dt)

        bqk_m = np.empty((64, 8, 1), np.float32)
        for ti_t, bb in enumerate((bq, bk)):
            bf = np.asarray(bb, np.float32)[sl] * QK_SCALE
            for chh in range(2):
                for hf in range(2):
                    js = hf * 128 + chh * 64 + np.arange(64)
                    bqk_m[:, 4 * ti_t + 2 * chh + hf, 0] = bf[colperm[js]]

        bv_m = np.concatenate([
            np.asarray(bv1, np.float32)[sl] * V_SCALE,
            np.asarray(bv2, np.float32)[sl] * V_SCALE]).reshape(1, 2 * CL)

        m = {
            "x1b": x1f[b].reshape(CT, 128, N).astype(_F8),
            "x2b": x2f[b].reshape(CT, 128, N).astype(_F8),
            "wqk": np.stack([wslice(Wq, QK_SCALE, colperm),
                             wslice(Wk, QK_SCALE, colperm)], axis=1),
            "wv": np.stack([wslice(Wv1, V_SCALE),
                            wslice(Wv2, V_SCALE)], axis=1),
            "wo": np.stack([wslice(Wo2, WO_SCALE),
                            wslice(Wo1, WO_SCALE)], axis=1),
            "bqk": bqk_m,
            "bv": bv_m.astype(_BF),
            "x1r": (x1f[b, sl, :] + np.asarray(bo1, np.float32)[sl, None]
                    ).reshape(2, 128, N),
            "x2r": (x2f[b, sl, :] + np.asarray(bo2, np.float32)[sl, None]
                    ).reshape(2, 128, N),
        }
        in_maps.append(m)
    return in_maps


def _unshard(res):
    o1 = np.empty((B, C, N), np.float32)
    o2 = np.empty((B, C, N), np.float32)
    for c in range(N_CORES):
        b, hq = c // 2, c % 2
        sl = slice(CL * hq, CL * hq + CL)
        o1[b, sl, :] = res[c]["o1"].reshape(CL, N)
        o2[b, sl, :] = res[c]["o2"].reshape(CL, N)
    shape = (B, C, 8, 16, 16)
    return o1.reshape(shape), o2.reshape(shape)


def kernel(**inputs):
    in_maps = _in_maps(**inputs)
    nc = _get_nc()
    res = run_bass_kernel_spmd(nc, in_maps, list(range(N_CORES))).results
    return _unshard(res)


# revision 57
# speedup vs baseline: 1.0735x; 1.0735x over previous
"""Trainium2 Bass kernel for the dual-softmax cross-attention module (fp8 V2).

Sharding: 8 cores = batch (4) x head-half (2).  Core c handles batch c//2 and
heads 4*(c%2) .. 4*(c%2)+4, producing a disjoint 256-channel slice of both
outputs after a 2-core AllGather of context halves.

Engine plan (per core):
- All hot matmuls run fp8e4m3 with DoubleRow perf mode (2 k-subtiles per
  instruction, 0.5 cycles/row): Q/K/V projections, scores, ctx1, ctx2.
- Host-side scaling keeps every fp8 tensor in the normal range: Wq/Wk x16
  (undone in the exp scale), Wv1/Wv2 x1024 (undone in Wo1/Wo2).
- exp(s) runs on the ACT engine in two [128,1024] chunks per query tile with
  fused accumulator rowsums (no DVE reduces).  E is stored fp8.
- E^T is produced by uint16 XBAR transposes of the fp8 E (adjacent k pairs
  land interleaved on one partition); ctx1 consumes the pair dim as its
  DoubleRow k-subtiles, with V1 projected in matching parity-interleaved
  token order (stride-2 token APs, free).
- ctx2 accumulates over query-tile pairs with a ones-column appended to the
  scaled V2 (colsum for free); rowsum/colsum reciprocals scale V instead of E.
- Per-head epilogue (ctx2 halves, colsum, ctx1 kb-bursts) runs interleaved
  into the next head's query loop, sized to exactly 8 PSUM banks.
"""

import sys

for _p in ("/opt/trn_rl_repo", "/opt/pypackages"):
    if _p not in sys.path:
        sys.path.insert(0, _p)

import numpy as np
import ml_dtypes

import concourse.bass as bass
import concourse.tile as tile
from concourse import bacc, mybir
from concourse.bass_utils import run_bass_kernel_spmd

F32 = mybir.dt.float32
BF16 = mybir.dt.bfloat16
FP8 = mybir.dt.float8e4
U16 = mybir.dt.uint16
AF = mybir.ActivationFunctionType
AX = mybir.AxisListType
DR = mybir.MatmulPerfMode.DoubleRow

N_CORES = 8
B = 4          # batch
C = 512        # channels
N = 2048       # tokens (8*16*16)
H = 8          # heads
DH = 64        # head dim
HL = 4         # heads per core
CL = 256       # channels per core (head-group)
NT = N // 128  # 16 token tiles
CT = C // 128  # 4 channel tiles
KB = 8         # k-blocks of 256 tokens (parity pairs on 128 partitions)

QK_SCALE = 16.0       # host scale on Wq/Wk (fp8 subnormal dodge)
V_SCALE = 1024.0      # host scale on Wv1/Wv2 (descaled in the output evac)
WO_SCALE = 8.0        # host scale on Wo1/Wo2 (fp8 subnormal dodge)
EXP_SCALE = 0.125 / (QK_SCALE * QK_SCALE)

_BF = ml_dtypes.bfloat16
_F8 = ml_dtypes.float8_e4m3


def _build():
    nc = bacc.Bacc("TRN2", target_bir_lowering=False, debug=False,
                   num_devices=N_CORES)

    def din(name, shape, dt=FP8):
        return nc.dram_tensor(name, shape, dt, kind="ExternalInput").ap()

    x1b = din("x1b", [CT, 128, N])          # x1[b] channel-major, fp8
    x2b = din("x2b", [CT, 128, N])
    wqk = din("wqk", [128, 2, CT, CL])      # Wq/Wk cols permuted, x16, fp8
    wv = din("wv", [128, 2, CT, CL])        # Wv1/Wv2 cols, x1024, fp8
    wo = din("wo", [128, 2, CT, CL])        # Wo2/Wo1 cols x8, fp8
    bqk = din("bqk", [64, 8, 1], F32)       # q/k bias, permuted, x16
    bv = din("bv", [1, 2 * CL], BF16)       # v1/v2 bias x1024, bf16
    x1r = din("x1r", [2, 128, N], F32)      # residual slice + bo1
    x2r = din("x2r", [2, 128, N], F32)

    o1 = nc.dram_tensor("o1", [2, 128, N], F32, kind="ExternalOutput").ap()
    o2 = nc.dram_tensor("o2", [2, 128, N], F32, kind="ExternalOutput").ap()

    with tile.TileContext(nc) as tc:
        _emit(nc, tc, locals())
    nc.compile()
    return nc


def _emit(nc, tc, t):
    x1b, x2b = t["x1b"], t["x2b"]
    wqk, wv, wo, bqk, bv = t["wqk"], t["wv"], t["wo"], t["bqk"], t["bv"]
    x1r, x2r, o1, o2 = t["x1r"], t["x2r"], t["o1"], t["o2"]

    from contextlib import ExitStack
    ctx = ExitStack()
    with ctx:
        persist = ctx.enter_context(tc.tile_pool(name="persist", bufs=1))
        small = ctx.enter_context(tc.tile_pool(name="small", bufs=8))
        dram = ctx.enter_context(tc.tile_pool(name="dram", bufs=2, space="DRAM"))

        # ---- persistent SBUF ----
        w8 = persist.tile([128, 4, CT, CL], FP8, tag="w8")       # q,k,v1,v2
        wo_s = persist.tile([128, 2, CT, CL], FP8, tag="wo")     # x8 scale
        bqk_s = persist.tile([64, 8, 1], F32, tag="bqk")
        bv_s = persist.tile([1, 2 * CL], BF16, tag="bv")
        bvb = persist.tile([128, 2 * CL], BF16, tag="bvb")  # broadcast rows
        onec = persist.tile([128, 1], BF16, tag="onec")  # bf16 ones col
        oner = persist.tile([1, 128], BF16, tag="oner")  # bf16 ones row
        # Q/K: [hl_loc*32+d32, tensor, head-pair, d-half, tok] (base 0/32)
        qk2 = persist.tile([64, 2, 2, 2, N], FP8, tag="qk2")
        v2tok = persist.tile([128, NT, CL], BF16, tag="v2tok")   # token-major
        v1tok = persist.tile([128, KB, 2, CL], BF16, tag="v1tok")  # parity
        et = persist.tile([128, KB, N], U16, tag="et")   # E^T, k-pairs per part
        cm = {}

        nc.gpsimd.dma_start(w8[:, 0:2, :, :], wqk[:, :, :, :])
        nc.gpsimd.dma_start(bqk_s[:, :, :], bqk[:, :, :])
        nc.gpsimd.dma_start(w8[:, 2:4, :, :], wv[:, :, :, :])
        nc.gpsimd.dma_start(bv_s[:, :], bv[:, :])
        nc.gpsimd.dma_start(wo_s[:, :, :, :], wo[:, :, :, :])
        nc.vector.memset(onec[:, :], 1.0)
        nc.vector.memset(oner[:, :], 1.0)

        # ---- P2 SBUF slab pools ----
        p2 = ExitStack()
        es_pool = p2.enter_context(tc.tile_pool(name="es", bufs=2))
        v2p_pool = p2.enter_context(tc.tile_pool(name="v2p", bufs=2))
        v1p_pool = p2.enter_context(tc.tile_pool(name="v1p", bufs=2))
        gs_pool = p2.enter_context(tc.tile_pool(name="gs", bufs=2))
        cs_pool = p2.enter_context(tc.tile_pool(name="cs", bufs=2))

        # ---- P1: x loads + Q/K projections (fp8 DR) ----
        xb_stack = ExitStack()
        xt_pool = xb_stack.enter_context(tc.tile_pool(name="xt", bufs=1))
        xt = xt_pool.tile([128, 2, CT, N], FP8, tag="xt")
        # x tiles on the scalar queue (idle until attention) so they overlap
        # the weight loads on gpsimd; ti 0,1 of both streams land first
        for ti in range(CT):
            for s, xsrc in ((0, x1b), (1, x2b)):
                nc.scalar.dma_start(xt[:, s, ti, :], xsrc[ti, :, :])

        pj_stack = ExitStack()
        pj_ps = pj_stack.enter_context(
            tc.tile_pool(name="pj_ps", bufs=4, space="PSUM"))
        for ti_t, s in ((0, 0), (1, 1)):
            for ch in range(2):
                for half in range(2):
                    for u in range(4):
                        ps = pj_ps.tile([64, 512], F32, tag="pj",
                                        name=f"pj{ti_t}_{ch}_{half}_{u}")
                        co = half * 128 + ch * 64
                        for tp in (0, 2):
                            nc.tensor.matmul(
                                ps[:, :],
                                w8[:, ti_t, tp:tp + 2, co:co + 64],
                                xt[:, s, tp:tp + 2, u * 512:(u + 1) * 512],
                                start=(tp == 0), stop=(tp == 2), perf_mode=DR)
                        nc.vector.tensor_scalar_add(
                            qk2[0:64, ti_t, ch, half, u * 512:(u + 1) * 512],
                            ps[:, :],
                            bqk_s[0:64, 4 * ti_t + 2 * ch + half, :])

        # broadcast bv to all 128 partitions via a K=1 matmul (emitted after
        # the Q/K projections so it doesn't block the PE queue head at start)
        bps = pj_ps.tile([128, 2 * CL], F32, tag="bvb", name="bvb_ps")
        nc.tensor.matmul(bps[:, :], oner[:, :], bv_s[:, :],
                         start=True, stop=True)
        nc.vector.tensor_copy(bvb[:, :], bps[:, :])
        pj_stack.close()

        # PSUM pools for P2 (opened only after pj frees its banks).
        # PSUM pools pop LIFO: sc_ps outlives vps, epi_ps opens after vps.
        sc_ps = p2.enter_context(tc.tile_pool(name="sc_ps", bufs=3, space="PSUM"))
        vps_stack = ExitStack()
        vps_pool = vps_stack.enter_context(
            tc.tile_pool(name="vps", bufs=2, space="PSUM"))
        epi_ps = None  # opened once vps closes (end of head 0)

        def emit_v2_proj(qt):
            ps = vps_pool.tile([128, 256], F32, tag="vps", name=f"v2ps{qt}")
            for tp in (0, 2):
                nc.tensor.matmul(
                    ps[:, 0:CL],
                    xt[:, 1, tp:tp + 2, qt * 128:(qt + 1) * 128],
                    w8[:, 3, tp:tp + 2, :],
                    start=(tp == 0), stop=(tp == 2), perf_mode=DR)
            nc.vector.tensor_add(v2tok[:, qt, :], ps[:, 0:CL],
                                 bvb[:, CL:2 * CL])

        def emit_v1_proj(kb, par):
            # parity-interleaved token order: partition j <- token kb*256+2j+par
            ps = vps_pool.tile([128, 256], F32, tag="vps", name=f"v1ps{kb}_{par}")
            xpar = xt[:, 0, :, :].rearrange(
                "p t (kb j two) -> p t kb two j", kb=KB, j=128, two=2)
            for tp in (0, 2):
                nc.tensor.matmul(
                    ps[:, 0:CL], xpar[:, tp:tp + 2, kb, par, :],
                    w8[:, 2, tp:tp + 2, :],
                    start=(tp == 0), stop=(tp == 2), perf_mode=DR)
            nc.vector.tensor_add(v1tok[:, kb, par, :], ps[:, 0:CL],
                                 bvb[:, 0:CL])

        # ---- P2: per-head attention, epilogue of head hl-1 interleaved ----
        st = {}

        def emit_scores_exp(hl, qt):
            s = st[hl]
            lo, hp = (hl % 2) * 32, hl // 2
            for c in range(2):
                ps = sc_ps.tile([128, 1024], F32, tag="sc", name=f"sc{qt}_{c}")
                for u in (2 * c, 2 * c + 1):
                    nc.tensor.matmul(
                        ps[:, (u % 2) * 512:(u % 2) * 512 + 512],
                        qk2[lo:lo + 32, 0, hp, :, qt * 128:(qt + 1) * 128],
                        qk2[lo:lo + 32, 1, hp, :, u * 512:(u + 1) * 512],
                        start=True, stop=True, perf_mode=DR)
                nc.scalar.activation(
                    s["es"][:, qt, c * 1024:(c + 1) * 1024], ps[:, :],
                    AF.Exp, scale=EXP_SCALE,
                    accum_out=s["rsp"][:, 2 * qt + c:2 * qt + c + 1])

        def emit_rownorm(hl, qt):
            s = st[hl]
            sq = small.tile([128, 8], F32, tag="sq", bufs=4, name=f"sq{hl}_{qt}")
            rs, rr = sq[:, 0:1], sq[:, 1:2]
            nc.vector.reduce_sum(out=rs[:, :], in_=s["rsp"][:, 2 * qt:2 * qt + 2],
                                 axis=AX.X)
            nc.vector.reciprocal(rr[:, :], rs[:, :])
            nc.vector.tensor_scalar_mul(
                s["v2p"][:, qt, 0:DH],
                v2tok[:, qt, hl * DH:(hl + 1) * DH], rr[:, :])

        def emit_ctx2(hl, pair, half):
            # 2 DR instrs accumulating a query-tile pair into a k-half
            s = st[hl]
            for ch in range(2):
                o = half * 1024 + ch * 512
                nc.tensor.matmul(
                    s["c2"][half][0:DH + 1, ch * 512:(ch + 1) * 512],
                    s["v2p"][:, 2 * pair:2 * pair + 2, 0:DH + 1],
                    s["es"][:, 2 * pair:2 * pair + 2, o:o + 512],
                    start=(pair == 0), stop=(pair == NT // 2 - 1), perf_mode=DR)

        def emit_ctx2_evac(hl, half):
            s = st[hl]
            nc.vector.tensor_copy(s["csrow"][0:1, half * 1024:(half + 1) * 1024],
                                  s["c2"][half][DH:DH + 1, :])
            nc.vector.tensor_copy(s["gs"][0:DH, half * 1024:(half + 1) * 1024],
                                  s["c2"][half][0:DH, :])

        def emit_colsum(hl):
            # colsum row -> parity-ordered columns via 16 K=1 matmuls
            s = st[hl]
            cs_ps = epi_ps.tile([128, 16], F32, tag="epi", name=f"csps{hl}")
            csp = s["csrow"].rearrange(
                "p (kb j two) -> p kb two j", kb=KB, j=128, two=2)
            for kb in range(KB):
                for par in range(2):
                    nc.tensor.matmul(cs_ps[:, 2 * kb + par:2 * kb + par + 1],
                                     csp[0:1, kb, par, :], onec[0:1, :],
                                     start=True, stop=True)
            cr = small.tile([128, 16], F32, tag="cr", bufs=2, name=f"cr{hl}")
            s["cr"] = cr
            nc.vector.reciprocal(cr[:, :], cs_ps[:, :])

        def emit_v1p(hl, kb):
            s = st[hl]
            for par in range(2):
                nc.vector.tensor_scalar_mul(
                    s["v1p"][:, kb, par, :],
                    v1tok[:, kb, par, hl * DH:(hl + 1) * DH],
                    s["cr"][:, 2 * kb + par:2 * kb + par + 1])

        # ctx1 over the u16-transposed E^T: even-parity tokens (byte 0 of each
        # u16 pair, 2B-aligned start) use DoubleRow with kb-region pairs
        # (16KB-aligned subtile stride); odd-parity tokens start at an odd
        # byte, which DoubleRow forbids, so they run as plain fp8 matmuls.
        et8v = et.bitcast(FP8).rearrange(
            "p (g kbs) (q two) -> p kbs g two q", g=2, kbs=KB // 2, two=2)
        et8k = et.bitcast(FP8).rearrange(
            "p kb (q two) -> p kb two q", two=2)

        def emit_ctx1(hl, qc, part):
            # part 0: even-parity DR (4 instrs) + odd kb 0-3; part 1: odd kb 4-7
            # Each qc accumulates in its own 1-bank psum tile, evacuated
            # immediately (keeps the epilogue pool at 2 banks so sc_ps can
            # triple-buffer).
            s = st[hl]
            v1pv = s["v1p"].rearrange("p (g kbs) two d -> p kbs g two d", g=2)
            if part == 0:
                s["c1"] = epi_ps.tile([DH, 512], F32, tag="epi",
                                      name=f"c1_{hl}_{qc}")
            dst = s["c1"][0:DH, :]
            if part == 0:
                for kbs in range(KB // 2):
                    nc.tensor.matmul(
                        dst, v1pv[:, kbs, :, 0, :],
                        et8v[:, kbs, :, 0, qc * 512:(qc + 1) * 512],
                        start=(kbs == 0), stop=False, perf_mode=DR)
                kbr = range(0, KB // 2)
            else:
                kbr = range(KB // 2, KB)
            for kb in kbr:
                nc.tensor.matmul(
                    dst, s["v1p"][:, kb, 1, :],
                    et8k[:, kb, 1, qc * 512:(qc + 1) * 512],
                    start=False, stop=(part == 1 and kb == KB - 1))
            if part == 1:
                nc.vector.tensor_copy(
                    s["gs"][DH:128, qc * 512:(qc + 1) * 512], dst)

        def emit_transpose(hl, qt):
            # one instr transposes all 8 kb chunks: out[p, kb, q] 3D AP.
            # Alternate queues so the per-head transpose chain halves.
            s = st[hl]
            eng = nc.sync
            eng.dma_start(
                et[:, :, qt * 128:(qt + 1) * 128],
                s["es"][:, qt, :].bitcast(U16),
                transpose=True)

        def emit_gather(hl, half=None):
            s = st[hl]
            poff = 64 * (hl % 2)
            rows = slice(0, 128) if half is None else (
                slice(0, 64) if half == 0 else slice(64, 128))
            nr = rows.stop - rows.start
            sfx = f"{hl}_{half}"
            gin = dram.tile([nr, N], FP8, tag="gin", name=f"gin{sfx}")
            gout = dram.tile([2, nr, N], FP8, tag="gout", bufs=4,
                             name=f"gout{sfx}")
            nc.gpsimd.dma_start(gin[:, :], s["gs"][rows, :])
            nc.gpsimd.collective_compute(
                "AllGather", mybir.AluOpType.bypass,
                replica_groups=[[0, 1], [2, 3], [4, 5], [6, 7]],
                ins=[gin.opt()], outs=[gout.opt()])
            for r in range(2):
                tt = 2 * r + hl // 2
                if half in (None, 0):
                    nc.gpsimd.dma_start(cm["2"][poff:poff + 64, tt, :],
                                        gout[r, 0:64, :])
                if half in (None, 1):
                    ro = 64 if half is None else 0
                    nc.gpsimd.dma_start(cm["1"][poff:poff + 64, tt, :],
                                        gout[r, ro:ro + 64, :])

        CTX2_SLOTS = {1: (0, 0, 3), 2: (3, 0, 3), 3: (6, 0, 2),
                      4: (0, 1, 3), 5: (3, 1, 3), 6: (6, 1, 2)}

        def emit_epilogue_piece(hl, qt):
            # epilogue of head hl scheduled into head hl+1's qt slot
            if qt in CTX2_SLOTS:
                p0, half, n = CTX2_SLOTS[qt]
                for pair in range(p0, p0 + n):
                    emit_ctx2(hl, pair, half)
                if p0 + n == KB:
                    emit_ctx2_evac(hl, half)
            elif qt == 7:
                emit_colsum(hl)
                for kb in range(KB):
                    emit_v1p(hl, kb)
            elif 8 <= qt <= 15:  # ctx1: 8 balanced pieces
                emit_ctx1(hl, (qt - 8) // 2, (qt - 8) % 2)

        def finish_ctx1(hl):
            emit_gather(hl)

        def new_head_state(hl):
            st[hl] = {
                "es": es_pool.tile([128, NT, N], FP8, tag="es", name=f"es{hl}"),
                "c2": {}, "c1": None, "cr": None,
                "rsp": small.tile([128, 32], F32, tag="rsp", bufs=2,
                                  name=f"rsp{hl}"),
                "gs": gs_pool.tile([128, N], FP8, tag="gs", name=f"gs{hl}"),
                "csrow": cs_pool.tile([1, N], BF16, tag="csr", name=f"csr{hl}"),
            }
            # 80B row stride: DoubleRow subtile strides must be 16B-aligned
            v2p = v2p_pool.tile([128, NT, 80], FP8, tag="v2p",
                                name=f"v2p{hl}")
            st[hl]["v2p"] = v2p
            nc.vector.memset(v2p[:, :, DH:DH + 1], 1.0)
            st[hl]["v1p"] = v1p_pool.tile([128, KB, 2, DH], FP8, tag="v1p",
                                          name=f"v1p{hl}")

        for hl in range(HL):
            new_head_state(hl)
            if hl > 0:
                st[hl - 1]["c2"][0] = epi_ps.tile([DH + 1, 1024], F32, tag="epi",
                                                  name=f"c2a{hl - 1}")
            for qt in range(NT):
                emit_scores_exp(hl, qt)
                if hl > 0:
                    emit_epilogue_piece(hl - 1, qt)
                    if qt == 3:
                        st[hl - 1]["c2"][1] = epi_ps.tile(
                            [DH + 1, 1024], F32, tag="epi", name=f"c2b{hl - 1}")
                else:
                    emit_v2_proj(qt)
                    emit_v1_proj(qt // 2, qt % 2)
                emit_rownorm(hl, qt)
                # own transposes start as ctx1(hl-1) frees et q-ranges:
                # tq 0..11 during slots 10..15, tq 12..15 spill to the next
                # head's slots 0..1 (or the post-loop for the last head)
                if qt >= 10:
                    emit_transpose(hl, 2 * (qt - 10))
                    emit_transpose(hl, 2 * (qt - 10) + 1)
                if hl > 0 and qt < 2:
                    emit_transpose(hl - 1, 12 + 2 * qt)
                    emit_transpose(hl - 1, 12 + 2 * qt + 1)
            if hl == 0:
                vps_stack.close()
                xb_stack.close()
                cm_pool = p2.enter_context(tc.tile_pool(name="cm", bufs=1))
                cm["1"] = cm_pool.tile([128, CT, N], FP8, tag="cm1", name="cm1")
                cm["2"] = cm_pool.tile([128, CT, N], FP8, tag="cm2", name="cm2")
                epi_ps = p2.enter_context(
                    tc.tile_pool(name="epi_ps", bufs=1, space="PSUM"))
            if hl > 0:
                finish_ctx1(hl - 1)

        # ---- last head's epilogue + transposes ----
        hl = HL - 1
        s = st[hl]
        s["c2"][0] = epi_ps.tile([DH + 1, 1024], F32, tag="epi", name=f"c2a{hl}")
        for pair in range(NT // 2):
            emit_ctx2(hl, pair, 0)
        emit_ctx2_evac(hl, 0)
        s["c2"][1] = epi_ps.tile([DH + 1, 1024], F32, tag="epi", name=f"c2b{hl}")
        for pair in range(NT // 2):
            emit_ctx2(hl, pair, 1)
        for tq in range(12, NT):
            emit_transpose(hl, tq)
        emit_ctx2_evac(hl, 1)
        emit_gather(hl, half=0)
        emit_colsum(hl)
        for kb in range(KB):
            emit_v1p(hl, kb)
        for qc in range(4):
            for part in range(2):
                emit_ctx1(hl, qc, part)
        emit_gather(hl, half=1)

        p2.close()

        # ---- P3: output projections + residual ----
        p3 = ExitStack()
        o_ps = p3.enter_context(tc.tile_pool(name="o_ps", bufs=2, space="PSUM"))
        xr_pool = p3.enter_context(tc.tile_pool(name="xr", bufs=2))
        out_pool = p3.enter_context(tc.tile_pool(name="outp", bufs=2))
        for oi, (cmt, xr, oo) in enumerate(((cm["2"], x2r, o2),
                                            (cm["1"], x1r, o1))):
            w_s = wo_s[:, oi, :, :]
            cmv = cmt.rearrange("p (g ti) n -> p ti g n", g=2)
            wv_ = w_s.rearrange("p (g ti) c -> p ti g c", g=2)
            for m in range(2):
                xr_t = xr_pool.tile([128, N], F32, tag="xr")
                nc.gpsimd.dma_start(xr_t[:, :], xr[m, :, :])
                ps = o_ps.tile([128, N], F32, tag="o")
                # DR over cm-tile pairs (0,2) then (1,3): heads 0-1/4-5 are
                # gathered two heads early, so the first half starts sooner
                for ti in (0, 1):
                    for ch in range(4):
                        nc.tensor.matmul(
                            ps[:, ch * 512:(ch + 1) * 512],
                            wv_[:, ti, :, m * 128:(m + 1) * 128],
                            cmv[:, ti, :, ch * 512:(ch + 1) * 512],
                            start=(ti == 0), stop=(ti == 1), perf_mode=DR)
                ot = out_pool.tile([128, N], F32, tag="ot")
                nc.vector.scalar_tensor_tensor(
                    ot[:, :], ps[:, :], 1.0 / (V_SCALE * WO_SCALE), xr_t[:, :],
                    op0=mybir.AluOpType.mult, op1=mybir.AluOpType.add)
                nc.scalar.dma_start(oo[m, :, :], ot[:, :])
        p3.close()


_NC_CACHE = None


def _get_nc():
    global _NC_CACHE
    if _NC_CACHE is None:
        _NC_CACHE = _build()
    return _NC_CACHE


def _in_maps(x1, x2, Wq, bq, Wk, bk, Wv1, bv1, Wv2, bv2, Wo1, bo1, Wo2, bo2):
    x1f = np.asarray(x1, np.float32).reshape(B, C, N)
    x2f = np.asarray(x2, np.float32).reshape(B, C, N)

    # permuted column order for Wq/Wk: j = half*128 + ch*64 + hl_loc*32 + d32
    j = np.arange(CL)
    half, r = j // 128, j % 128
    chp, r2 = r // 64, r % 64
    colperm = (2 * chp + r2 // 32) * 64 + half * 32 + (r2 % 32)

    in_maps = []
    for c in range(N_CORES):
        b, hq = c // 2, c % 2
        sl = slice(CL * hq, CL * hq + CL)

        def wslice(W, scale, perm=None, dt=_F8):
            w = np.asarray(W, np.float32)[:, sl] * scale
            if perm is not None:
                w = w[:, perm]
            return np.ascontiguousarray(
                w.reshape(CT, 128, CL).transpose(1, 0, 2)).astype(dt)

        bqk_m = np.empty((64, 8, 1), np.float32)
        for ti_t, bb in enumerate((bq, bk)):
            bf = np.asarray(bb, np.float32)[sl] * QK_SCALE
            for chh in range(2):
                for hf in range(2):
                    js = hf * 128 + chh * 64 + np.arange(64)
                    bqk_m[:, 4 * ti_t + 2 * chh + hf, 0] = bf[colperm[js]]

        bv_m = np.concatenate([
            np.asarray(bv1, np.float32)[sl] * V_SCALE,
            np.asarray(bv2, np.float32)[sl] * V_SCALE]).reshape(1, 2 * CL)

        m = {
            "x1b": x1f[b].reshape(CT, 128, N).astype(_F8),
            "x2b": x2f[b].reshape(CT, 128, N).astype(_F8),
            "wqk": np.stack([wslice(Wq, QK_SCALE, colperm),
                             wslice(Wk, QK_SCALE, colperm)], axis=1),
            "wv": np.stack([wslice(Wv1, V_SCALE),
                            wslice(Wv2, V_SCALE)], axis=1),
            "wo": np.stack([wslice(Wo2, WO_SCALE),
                            wslice(Wo1, WO_SCALE)], axis=1),
            "bqk": bqk_m,
            "bv": bv_m.astype(_BF),
            "x1r": (x1f[b, sl, :] + np.asarray(bo1, np.float32)[sl, None]
                    ).reshape(2, 128, N),
            "x2r": (x2f[b, sl, :] + np.asarray(bo2, np.float32)[sl, None]
                    ).reshape(2, 128, N),
        }
        in_maps.append(m)
    return in_maps


def _unshard(res):
    o1 = np.empty((B, C, N), np.float32)
    o2 = np.empty((B, C, N), np.float32)
    for c in range(N_CORES):
        b, hq = c // 2, c % 2
        sl = slice(CL * hq, CL * hq + CL)
        o1[b, sl, :] = res[c]["o1"].reshape(CL, N)
        o2[b, sl, :] = res[c]["o2"].reshape(CL, N)
    shape = (B, C, 8, 16, 16)
    return o1.reshape(shape), o2.reshape(shape)


def kernel(**inputs):
    in_maps = _in_maps(**inputs)
    nc = _get_nc()
    res = run_bass_kernel_spmd(nc, in_maps, list(range(N_CORES))).results
    return _unshard(res)
